# revision 6
# baseline (speedup 1.0000x reference)
"""MoE layer (E=8, top-2, SwiGLU) Trainium2 Bass kernel.

Strategy: expert-parallel over 8 NeuronCores. Core e holds expert e's
weights (bf16). Every core redundantly computes the fp32 gate path
(logits, noisy top-2, gates, load-balance loss) — it is tiny and keeps
topk bit-exact vs the fp32 reference. Each core then runs its expert
densely over all T tokens (gates are 0 off top-k, so the gate-weighted
partial output is exact), scales by its expert's gate column, and a
ReduceScatter(add) sums partials across cores; the host concatenates
the 8 shards.
"""

import numpy as np
import ml_dtypes

E = 8
KTOP = 2
D = 768
H = 3072
T = 2048
NCORES = 8
PD = D // 128   # 6 k-chunks over D
PH = H // 128   # 24 k-chunks over H
TCH = 512       # token chunk for the expert phase
NCH = T // TCH  # 4
NT = T // 128   # 16 token tiles
SH = T // NCORES  # 256 rows per reduce-scatter shard
DH = 384        # D split in two 384 halves for PSUM-bank-sized matmuls

_BUILT = {}


def _build_program():
    import concourse.bass as bass
    import concourse.mybir as mybir
    import concourse.tile as tile
    from concourse import bacc

    f32 = mybir.dt.float32
    bf16 = mybir.dt.bfloat16
    i32 = mybir.dt.int32
    u32 = mybir.dt.uint32
    Alu = mybir.AluOpType
    Act = mybir.ActivationFunctionType

    nc = bacc.Bacc("TRN2", num_devices=NCORES, debug=False)

    xf = nc.dram_tensor("xf", [D, T], f32, kind="ExternalInput").ap()
    xb = nc.dram_tensor("xb", [D, T], bf16, kind="ExternalInput").ap()
    noise = nc.dram_tensor("noise", [T, E], f32, kind="ExternalInput").ap()
    gw = nc.dram_tensor("gw", [D, E], f32, kind="ExternalInput").ap()
    nw = nc.dram_tensor("nw", [E], f32, kind="ExternalInput").ap()
    w1 = nc.dram_tensor("w1", [D, H], bf16, kind="ExternalInput").ap()
    w2 = nc.dram_tensor("w2", [D, H], bf16, kind="ExternalInput").ap()
    wp = nc.dram_tensor("wp", [H, D], bf16, kind="ExternalInput").ap()
    b1 = nc.dram_tensor("b1", [128, PH], f32, kind="ExternalInput").ap()
    b2 = nc.dram_tensor("b2", [128, PH], f32, kind="ExternalInput").ap()
    bp = nc.dram_tensor("bp", [D], f32, kind="ExternalInput").ap()
    ecol = nc.dram_tensor("ecol", [1], f32, kind="ExternalInput").ap()

    out_shard = nc.dram_tensor("out_shard", [SH, D], f32, kind="ExternalOutput").ap()
    ids_out = nc.dram_tensor("ids_out", [T, KTOP], i32, kind="ExternalOutput").ap()
    lb_out = nc.dram_tensor("lb_out", [1], f32, kind="ExternalOutput").ap()

    xf_t = xf.rearrange("(k p) t -> p k t", p=128)   # [128, PD, T]
    xb_t = xb.rearrange("(k p) t -> p k t", p=128)

    with tile.TileContext(nc, num_cores=NCORES) as tc:
        with (
            tc.tile_pool(name="singles", bufs=1) as singles,
            tc.tile_pool(name="gx", bufs=2) as gx,
            tc.tile_pool(name="gsm", bufs=3) as gsm,
            tc.tile_pool(name="xp", bufs=2) as xp,
            tc.tile_pool(name="actp", bufs=1) as actp,
            tc.tile_pool(name="yp", bufs=3) as yp,
            tc.tile_pool(name="psg", bufs=1, space="PSUM") as psg,
            tc.tile_pool(name="hgp", bufs=2, space="PSUM") as hgp,
            tc.tile_pool(name="pyp", bufs=2, space="PSUM") as pyp,
            tc.tile_pool(name="dramp", bufs=1, space="DRAM") as dramp,
        ):
            # ---- resident tiles ----
            w1_sb = singles.tile([128, PD, H], bf16)
            w2_sb = singles.tile([128, PD, H], bf16)
            wp_sb = singles.tile([128, PH, D], bf16)
            for k in range(PD):
                nc.sync.dma_start(out=w1_sb[:, k, :], in_=w1[k * 128:(k + 1) * 128, :])
                nc.sync.dma_start(out=w2_sb[:, k, :], in_=w2[k * 128:(k + 1) * 128, :])
            for k in range(PH):
                nc.sync.dma_start(out=wp_sb[:, k, :], in_=wp[k * 128:(k + 1) * 128, :])

            gw_sb = singles.tile([128, PD, E], f32)
            for k in range(PD):
                nc.sync.dma_start(out=gw_sb[:, k, :], in_=gw[k * 128:(k + 1) * 128, :])
            b1_sb = singles.tile([128, PH], f32)
            b2_sb = singles.tile([128, PH], f32)
            nc.sync.dma_start(out=b1_sb, in_=b1)
            nc.sync.dma_start(out=b2_sb, in_=b2)
            bp_sb = singles.tile([128, D], f32)
            nc.sync.dma_start(out=bp_sb, in_=bp.partition_broadcast(128))
            nw_sb = singles.tile([128, E], f32)
            nc.sync.dma_start(out=nw_sb, in_=nw.partition_broadcast(128))
            ec_sb = singles.tile([128, 1], f32)
            nc.sync.dma_start(out=ec_sb, in_=ecol.partition_broadcast(128))
            ones_sb = singles.tile([128, 1], f32)
            nc.vector.memset(ones_sb, 1.0)

            gcol = singles.tile([128, NT], f32)     # this expert's gate, col per tok-tile
            pacc = singles.tile([128, E], f32)      # softmax(clean logits) accumulator
            nc.vector.memset(pacc, 0.0)

            partial = dramp.tile([T, D], f32)
            rs_out = dramp.tile([SH, D], f32)

            # ---- gate phase: 16 token tiles of 128 ----
            for m in range(NT):
                tsl = slice(m * 128, (m + 1) * 128)
                xg = gx.tile([128, PD, 128], f32)
                nc.sync.dma_start(out=xg, in_=xf_t[:, :, tsl])
                ps = psg.tile([128, E], f32)
                for k in range(PD):
                    nc.tensor.matmul(ps, lhsT=xg[:, k, :], rhs=gw_sb[:, k, :],
                                     start=(k == 0), stop=(k == PD - 1))
                lg = gsm.tile([128, E], f32)
                nc.vector.tensor_copy(out=lg, in_=ps)

                # load-balance softmax on clean logits (no max-sub needed;
                # logits are O(1) so exp can't overflow)
                ex = gsm.tile([128, E], f32)
                nc.scalar.activation(ex, lg, Act.Exp)
                sm = gsm.tile([128, 1], f32)
                nc.vector.reduce_sum(sm, ex, axis=mybir.AxisListType.X)
                rc = gsm.tile([128, 1], f32)
                nc.vector.reciprocal(rc, sm)
                pr = gsm.tile([128, E], f32)
                nc.vector.tensor_scalar_mul(pr, ex, rc)
                nc.vector.tensor_add(pacc, pacc, pr)

                # noisy logits + top-2
                nz = gsm.tile([128, E], f32)
                nc.sync.dma_start(out=nz, in_=noise[tsl, :])
                nc.vector.tensor_mul(nz, nz, nw_sb)
                nc.vector.tensor_add(nz, nz, lg)
                v8 = gsm.tile([128, 8], f32)
                nc.vector.max(v8, nz)
                i8 = gsm.tile([128, 8], u32)
                nc.vector.max_index(i8, v8, nz)

                idt = gsm.tile([128, KTOP], i32)
                nc.vector.tensor_copy(out=idt, in_=i8[:, 0:KTOP].bitcast(i32))
                nc.sync.dma_start(out=ids_out[tsl, :], in_=idt)

                # gates: g1 = sigmoid(v1 - v2), g2 = 1 - g1
                d12 = gsm.tile([128, 1], f32)
                nc.vector.tensor_sub(d12, v8[:, 0:1], v8[:, 1:2])
                g1 = gsm.tile([128, 1], f32)
                nc.scalar.activation(g1, d12, Act.Sigmoid)
                g2 = gsm.tile([128, 1], f32)
                nc.vector.tensor_scalar(out=g2, in0=g1, scalar1=-1.0, scalar2=1.0,
                                        op0=Alu.mult, op1=Alu.add)
                idf = gsm.tile([128, KTOP], f32)
                nc.vector.tensor_copy(out=idf, in_=i8[:, 0:KTOP])
                eq1 = gsm.tile([128, 1], f32)
                nc.vector.tensor_tensor(out=eq1, in0=idf[:, 0:1], in1=ec_sb,
                                        op=Alu.is_equal)
                eq2 = gsm.tile([128, 1], f32)
                nc.vector.tensor_tensor(out=eq2, in0=idf[:, 1:2], in1=ec_sb,
                                        op=Alu.is_equal)
                nc.vector.tensor_mul(eq1, eq1, g1)
                nc.vector.tensor_mul(eq2, eq2, g2)
                nc.vector.tensor_add(gcol[:, m:m + 1], eq1, eq2)

            # ---- lb loss ----
            lbs = psg.tile([1, E], f32)
            nc.tensor.matmul(lbs, lhsT=ones_sb, rhs=pacc, start=True, stop=True)
            gm = gsm.tile([1, E], f32)
            nc.vector.tensor_scalar(out=gm, in0=lbs, scalar1=1.0 / T,
                                    scalar2=-1.0 / E, op0=Alu.mult, op1=Alu.add)
            nc.vector.tensor_mul(gm, gm, gm)
            lbv = gsm.tile([1, 1], f32)
            nc.vector.reduce_sum(lbv, gm, axis=mybir.AxisListType.X)
            nc.vector.tensor_scalar_mul(lbv, lbv, 0.01 / E)
            nc.sync.dma_start(out=lb_out, in_=lbv)

            # ---- expert phase: 4 chunks of 512 tokens ----
            for c in range(NCH):
                csl = slice(c * TCH, (c + 1) * TCH)
                xbc = xp.tile([128, PD, TCH], bf16)
                nc.sync.dma_start(out=xbc, in_=xb_t[:, :, csl])
                act = actp.tile([128, PH, TCH], bf16)
                for mh in range(PH):
                    hsl = slice(mh * 128, (mh + 1) * 128)
                    ph = hgp.tile([128, TCH], f32)
                    pg = hgp.tile([128, TCH], f32)
                    for k in range(PD):
                        nc.tensor.matmul(ph, lhsT=w1_sb[:, k, hsl], rhs=xbc[:, k, :],
                                         start=(k == 0), stop=(k == PD - 1))
                    for k in range(PD):
                        nc.tensor.matmul(pg, lhsT=w2_sb[:, k, hsl], rhs=xbc[:, k, :],
                                         start=(k == 0), stop=(k == PD - 1))
                    sg = gsm.tile([128, TCH], f32)
                    nc.scalar.activation(sg, pg, Act.Sigmoid, bias=b2_sb[:, mh:mh + 1])
                    gb = gsm.tile([128, TCH], f32)
                    # silu(g + b2) = (g + b2) * sigmoid(g + b2)
                    nc.vector.scalar_tensor_tensor(
                        out=gb, in0=pg, scalar=b2_sb[:, mh:mh + 1],
                        in1=sg, op0=Alu.add, op1=Alu.mult)
                    # act = (h + b1) * silu(g + b2), cast to bf16
                    nc.vector.scalar_tensor_tensor(
                        out=act[:, mh, :], in0=ph, scalar=b1_sb[:, mh:mh + 1],
                        in1=gb, op0=Alu.add, op1=Alu.mult)
                for mt in range(TCH // 128):
                    gidx = c * (TCH // 128) + mt
                    tsl2 = slice(mt * 128, (mt + 1) * 128)
                    for n2 in range(2):
                        dsl = slice(n2 * DH, (n2 + 1) * DH)
                        py = pyp.tile([128, DH], f32)
                        for k in range(PH):
                            nc.tensor.matmul(py, lhsT=act[:, k, tsl2],
                                             rhs=wp_sb[:, k, dsl],
                                             start=(k == 0), stop=(k == PH - 1))
                        yt = yp.tile([128, DH], f32)
                        nc.vector.tensor_add(yt, py, bp_sb[:, dsl])
                        nc.vector.tensor_scalar_mul(yt, yt, gcol[:, gidx:gidx + 1])
                        nc.sync.dma_start(
                            out=partial[gidx * 128:(gidx + 1) * 128, dsl], in_=yt)

            # ---- combine across cores ----
            nc.gpsimd.collective_compute(
                "ReduceScatter", mybir.AluOpType.add,
                replica_groups=[list(range(NCORES))],
                ins=[partial.opt()], outs=[rs_out.opt()])
            nc.sync.dma_start(out=out_shard, in_=rs_out)

    nc.compile()
    return nc


def _get_program():
    if "nc" not in _BUILT:
        _BUILT["nc"] = _build_program()
    return _BUILT["nc"]


def _prep_in_maps(inputs):
    bf16 = ml_dtypes.bfloat16
    x = np.asarray(inputs["x"], np.float32).reshape(T, D)
    xf = np.ascontiguousarray(x.T)
    xb = xf.astype(bf16)
    noise = np.ascontiguousarray(np.asarray(inputs["noise"], np.float32))
    gw = np.ascontiguousarray(np.asarray(inputs["gate_w"], np.float32))
    nw = np.ascontiguousarray(np.asarray(inputs["noise_weight"], np.float32))
    w1 = np.asarray(inputs["w1"], np.float32)
    w2 = np.asarray(inputs["w2"], np.float32)
    wpm = np.asarray(inputs["wp"], np.float32)
    b1 = np.asarray(inputs["b1"], np.float32)
    b2 = np.asarray(inputs["b2"], np.float32)
    bpm = np.asarray(inputs["bp"], np.float32)

    in_maps = []
    for e in range(NCORES):
        in_maps.append({
            "xf": xf,
            "xb": xb,
            "noise": noise,
            "gw": gw,
            "nw": nw,
            "w1": np.ascontiguousarray(w1[e]).astype(bf16),
            "w2": np.ascontiguousarray(w2[e]).astype(bf16),
            "wp": np.ascontiguousarray(wpm[e]).astype(bf16),
            "b1": np.ascontiguousarray(b1[e].reshape(PH, 128).T),
            "b2": np.ascontiguousarray(b2[e].reshape(PH, 128).T),
            "bp": np.ascontiguousarray(bpm[e]),
            "ecol": np.full([1], e, np.float32),
        })
    return in_maps


def kernel(**inputs):
    from concourse.bass_utils import run_bass_kernel_spmd

    nc = _get_program()
    in_maps = _prep_in_maps(inputs)
    res = run_bass_kernel_spmd(nc, in_maps, list(range(NCORES)))
    out = np.concatenate([res.results[c]["out_shard"] for c in range(NCORES)], axis=0)
    out = out.reshape(2, 1024, D)
    ids = np.asarray(res.results[0]["ids_out"], np.int32)
    lb = np.float32(res.results[0]["lb_out"][0])
    return out, ids, lb
